# revision 6
# baseline (speedup 1.0000x reference)
"""Trainium2 Bass kernel for a single causal-attention transformer block.

fp8(e4m3) + DoubleRow rewrite of the bf16 baseline.  Reference computation
per batch element b:
    xn  = rms_norm(x[b]) * rms_w
    q/k/v = xn @ Wq/Wk/Wv            (16 heads x 128 head dim)
    att = causal_softmax(q k^T / sqrt(2048)) @ v
    out[b] = att @ Wo + x[b]

Sharding (8 NeuronCores): tensor-parallel over heads x data-parallel over
batch.  Core c handles batch b = c // 4 and head-group i = c % 4 (4 heads,
512 columns of Wq/Wk/Wv, 512 rows of Wo).  Each core computes a partial
output  att_i @ Wo_i * 64  for its batch element; the host sums the 4
partials per batch (/64) and adds the residual.

All big matmuls use fp8e4 with MatmulPerfMode.DoubleRow (2 k-tiles of 128
contracted per instruction = 2x bf16 MAC throughput on TRN2).  Contraction
dims are pre-paired in SBUF: x^T and the weights as [128,2,*] pair tiles
(host packs rows 256p..256p+255 together), probs/v paired over adjacent
key-tiles, attn^T paired over heads.  Scores (contract dim = head dim 128)
cannot pair and stay bf16.

The PE executes its instruction stream in order, so emission order is the
schedule: k-head-group projections are emitted first (paced by the x/w DMA
arrivals) with the RMS sum-of-squares matmuls woven in, v-projection
s-tile blocks sit between q projections, and each attention chunk's
o_proj waves are emitted one-per-probs-tile inside the NEXT chunk's
attention stream so their PSUM evacuations never stall the PE.

Precision: weights pre-scaled on host by powers of 2 into e4m3 range
(Wq/Wk x HID**-.25 * 512, Wv/Wo x 64); inverse scales fold into the rstd
evacuation multipliers and the host-side gather (/64).
"""

import numpy as np
import ml_dtypes

S = 2048          # sequence length
HID = 2048        # hidden dim
KSH = 512         # per-core key-dim shard
DH = 128          # head dim
NHS = 4           # heads per core
TP = 4            # head-group shards
DP = 2            # batch shards
NP = 8            # hidden-dim pair tiles (256 rows each)
NT = S // 128     # 16
NSC = S // 512    # 4
EPS = 1e-5
SQK = 512.0       # host pre-scale on Wq,Wk (each also carries HID**-0.25)
SVO = 64.0        # host pre-scale on Wv and Wo

_STATE = {}


def _build_nc():
    from contextlib import ExitStack

    import concourse.bacc as bacc
    import concourse.tile as tile
    from concourse import mybir

    F32 = mybir.dt.float32
    BF = mybir.dt.bfloat16
    FP8 = mybir.dt.float8e4
    AF = mybir.ActivationFunctionType
    DR = mybir.MatmulPerfMode.DoubleRow

    nc = bacc.Bacc("TRN2")
    xp_d = nc.dram_tensor("xp", [NP * 128, 2 * S], FP8, kind="ExternalInput")
    wq_d = nc.dram_tensor("wq", [NP * 128, 2 * KSH], FP8, kind="ExternalInput")
    wk_d = nc.dram_tensor("wk", [NP * 128, 2 * KSH], FP8, kind="ExternalInput")
    wv_d = nc.dram_tensor("wv", [NP * 128, 2 * KSH], FP8, kind="ExternalInput")
    wo_d = nc.dram_tensor("wo", [2 * 128, 2 * HID], FP8, kind="ExternalInput")
    out = nc.dram_tensor("out", [S, HID], BF, kind="ExternalOutput")

    with tile.TileContext(nc) as tc, ExitStack() as ctx:
        misc = ctx.enter_context(tc.tile_pool(name="misc", bufs=1))
        qt_pool = ctx.enter_context(tc.tile_pool(name="qt", bufs=NHS))
        kt_pool = ctx.enter_context(tc.tile_pool(name="kt", bufs=NHS))
        v_pool = ctx.enter_context(tc.tile_pool(name="v", bufs=NT // 2))
        at_pool = ctx.enter_context(tc.tile_pool(name="attn", bufs=2))
        pt_pool = ctx.enter_context(tc.tile_pool(name="probs", bufs=12))
        denb_pool = ctx.enter_context(tc.tile_pool(name="denb", bufs=4))
        wo_pool = ctx.enter_context(tc.tile_pool(name="wo", bufs=2))
        out_pool = ctx.enter_context(tc.tile_pool(name="outp", bufs=4))

        ones8 = misc.tile([128, 2, 128], FP8, tag="ones8", name="ones8")
        nc.vector.memset(ones8, 1.0)
        eps_sb = misc.tile([128, 1], F32, tag="eps_sb", name="eps_sb")
        nc.vector.memset(eps_sb, EPS * SQK * SQK)
        # rstd_b[p, s] = rstd[s]/SQK on every partition p (free-axis layout)
        rstd_b = misc.tile([128, S], F32, tag="rstd_b", name="rstd_b")
        # rstd_colT[p, st] = rstd[st*128+p]/SVO (partition-axis layout)
        rstd_colT = misc.tile([128, NT], F32, tag="rstd_colT", name="rstd_colT")
        ident = misc.tile([128, 128], F32, tag="ident", name="ident")
        nc.vector.memset(ident, 1.0)
        nc.gpsimd.affine_select(
            out=ident, in_=ident, compare_op=mybir.AluOpType.is_equal,
            fill=0.0, base=0, channel_multiplier=1, pattern=[[-1, 128]],
        )

        # PSUM: 2x [128,1024] (4 banks: projection dsts / score pairs) +
        # 4x [128,512] (ss, transposes, v, pv/den accums, o_proj waves)
        pp2 = ctx.enter_context(tc.tile_pool(name="pp2", bufs=2, space="PSUM"))
        pp = ctx.enter_context(tc.tile_pool(name="pp", bufs=4, space="PSUM"))

        # ---------------- phases A+B (x^T pairs + rstd + projections) -------
        # (x/w/square pools stay alive through phase C so leftover
        # v-projection tiles can fill the first attention chunk's bubbles)
        if True:
            xp_pool = ctx.enter_context(
                tc.tile_pool(name="xp", bufs=NP, side="right")
            )
            sqa_pool = ctx.enter_context(
                tc.tile_pool(name="sqa", bufs=3, side="right")
            )
            sqb_pool = ctx.enter_context(
                tc.tile_pool(name="sqb", bufs=NP, side="right")
            )
            w_pool = ctx.enter_context(
                tc.tile_pool(name="wstream", bufs=3 * NP + 2, side="right")
            )

            # --- DMA issue order: (xp[p], wk[p]) interleaved, then wq, wv, wo
            xp, wkts, sqas, sqbs = [], [], [], []
            sq_eng = [nc.vector, nc.scalar]
            nsq = 0
            for p in range(NP):
                t = xp_pool.tile([128, 2, S], FP8, tag="xp", name="xp")
                nsplit = 4 if p == 0 else 1
                step = 2 * S // nsplit
                for h in range(nsplit):
                    nc.sync.dma_start(
                        out=t[:, h * step // S:(h * step // S) + 1,
                              (h * step) % S:((h * step) % S) + step]
                        if step <= S else t[:, :, :],
                        in_=xp_d[p * 128:(p + 1) * 128,
                                 h * step:(h + 1) * step],
                    )
                xp.append(t)
                wt = w_pool.tile([128, 2, KSH], FP8, tag="w", name="wk")
                nc.sync.dma_start(out=wt, in_=wk_d[p * 128:(p + 1) * 128, :])
                wkts.append(wt)
                # squares in [128,2,1024] halves; engines rotate DVE/ACT/Pool
                sa = sqa_pool.tile([128, 2, 1024], FP8, tag="sqa", name="sqa")
                sb = sqb_pool.tile([128, 2, 1024], FP8, tag="sqb", name="sqb")
                for (dst_t, lo) in ((sa, 0), (sb, 1024)):
                    for i in range(2):
                        e = sq_eng[nsq % len(sq_eng)]
                        nsq += 1
                        if e is nc.scalar:
                            e.activation(
                                dst_t[:, i, :], t[:, i, lo:lo + 1024], AF.Square
                            )
                        else:
                            e.tensor_mul(
                                dst_t[:, i, :], t[:, i, lo:lo + 1024],
                                t[:, i, lo:lo + 1024],
                            )
                # (squares alternate DVE/ACT; Pool stays off the rstd path)
                sqas.append(sa)
                sqbs.append(sb)
            wqts, wvts = [], []
            for (lst, dram, nmw) in ((wqts, wq_d, "wq"), (wvts, wv_d, "wv")):
                for p in range(NP):
                    wt = w_pool.tile([128, 2, KSH], FP8, tag="w", name=nmw)
                    nc.sync.dma_start(out=wt, in_=dram[p * 128:(p + 1) * 128, :])
                    lst.append(wt)
            wo_sb = []
            for hp in range(2):
                wt = wo_pool.tile([128, 2, HID], FP8, tag="wo", name="wo")
                nc.sync.dma_start(out=wt, in_=wo_d[hp * 128:(hp + 1) * 128, :])
                wo_sb.append(wt)

            # --- PE stream helpers ------------------------------------------
            ss_ps = {}

            def emit_ss_pass(which):
                # two 512-chunks of the sum-of-squares accumulate over all
                # pairs; every PSUM partition gets the column sum (DR ones)
                srcs = sqas if which == 0 else sqbs
                ps = [pp.tile([128, 512], F32, tag="pp", name="ssp")
                      for _ in range(2)]
                ss_ps[which] = ps
                for p in range(NP):
                    for c in range(2):
                        nc.tensor.matmul(
                            ps[c], ones8, srcs[p][:, :, c * 512:(c + 1) * 512],
                            start=(p == 0), stop=(p == NP - 1), perf_mode=DR,
                        )

            def emit_rstd(which):
                for c in range(2):
                    g = 2 * which + c
                    cs = slice(g * 512, (g + 1) * 512)
                    mtmp = denb_pool.tile([128, 512], F32, tag="denb", name="mt")
                    nc.scalar.activation(
                        mtmp, ss_ps[which][c], AF.Sqrt,
                        bias=eps_sb, scale=SQK * SQK / HID,
                    )
                    nc.vector.reciprocal_approx_fast(rstd_b[:, cs], mtmp)

            def emit_dst_half(wts, dt, ps, half):
                # one 1024-column sweep of a projection head-block over the
                # 8 pairs
                for p in range(NP):
                    lhsT = wts[p][:, :, dt * 128:(dt + 1) * 128]
                    for c in range(2):
                        g = 2 * half + c
                        nc.tensor.matmul(
                            ps[half][:, c * 512:(c + 1) * 512],
                            lhsT,
                            xp[p][:, :, g * 512:(g + 1) * 512],
                            start=(p == 0), stop=(p == NP - 1),
                            perf_mode=DR,
                        )

            def emit_dst_evac_half(ps, dst, half):
                cs = slice(half * 1024, (half + 1) * 1024)
                nc.vector.tensor_mul(dst[:, cs], ps[half], rstd_b[:, cs])

            def emit_dst(wts, dst_pool, nmd, dt):
                # per-half evacuations free each PSUM ring slot one sweep
                # early, so the next dst's first sweep never stalls
                dst = dst_pool.tile([128, S], BF, tag="qt", name=nmd)
                ps = [pp2.tile([128, 1024], F32, tag="pp2", name="psqk")
                      for _ in range(2)]
                emit_dst_half(wts, dt, ps, 0)
                emit_dst_evac_half(ps, dst, 0)
                emit_dst_half(wts, dt, ps, 1)
                emit_dst_evac_half(ps, dst, 1)
                return dst

            def emit_transposes():
                # rstd -> per-partition layout (x SQK/SVO)
                for st in range(NT):
                    ptr = pp.tile([128, 512], F32, tag="pp", name="ptr")
                    nc.tensor.transpose(
                        ptr[:, 0:128], rstd_b[:, st * 128:(st + 1) * 128], ident
                    )
                    nc.vector.tensor_scalar_mul(
                        rstd_colT[:, st:st + 1], ptr[:, 0:1], SQK / SVO
                    )

            def emit_v(st):
                psv = pp.tile([128, 512], F32, tag="pp", name="psv")
                for p in range(NP):
                    nc.tensor.matmul(
                        psv, xp[p][:, :, st * 128:(st + 1) * 128], wvts[p],
                        start=(p == 0), stop=(p == NP - 1), perf_mode=DR,
                    )
                nc.vector.tensor_scalar_mul(
                    v_sb[st // 2][:, st % 2, :], psv, rstd_colT[:, st:st + 1]
                )

            # --- PE emission order ------------------------------------------
            # k0's two sweeps interleave with the two sum-of-squares passes
            # (all paced by the x DMA / squares), then rstd resolves while
            # the transposes fill the PE; k0's evacuations release the pp2
            # ring and every later dst streams densely.
            qts, kts = [], []
            v_sb = [v_pool.tile([128, 2, KSH], FP8, tag="v", name="v")
                    for _ in range(NT // 2)]
            k0 = kt_pool.tile([128, S], BF, tag="qt", name="kt")
            k0_ps = [pp2.tile([128, 1024], F32, tag="pp2", name="psqk")
                     for _ in range(2)]
            emit_dst_half(wkts, 0, k0_ps, 0)
            emit_ss_pass(0)
            emit_dst_half(wkts, 0, k0_ps, 1)
            emit_ss_pass(1)
            emit_rstd(0)
            emit_rstd(1)
            emit_transposes()
            emit_dst_evac_half(k0_ps, k0, 0)
            emit_dst_evac_half(k0_ps, k0, 1)
            kts.append(k0)
            kts.append(emit_dst(wkts, kt_pool, "kt", 1))
            kts.append(emit_dst(wkts, kt_pool, "kt", 2))
            kts.append(emit_dst(wkts, kt_pool, "kt", 3))
            qts.append(emit_dst(wqts, qt_pool, "qt", 0))
            for st in range(0, 4):
                emit_v(st)
            qts.append(emit_dst(wqts, qt_pool, "qt", 1))
            for st in range(4, 8):
                emit_v(st)
            qts.append(emit_dst(wqts, qt_pool, "qt", 2))
            for st in range(8, 12):
                emit_v(st)
            qts.append(emit_dst(wqts, qt_pool, "qt", 3))
            for st in range(12, 14):
                emit_v(st)
            v_rest = [st for st in range(14, NT)]
        # xp/sq/wstream released here

        # -------- phases C+D: attention + interleaved o_proj ---------------
        # attn^T head-pair tiles [dh, 2, S] fp8 (o_proj DR stationary)
        at_pair = [
            at_pool.tile([128, 2, S], FP8, tag="attn", name="attn")
            for _ in range(2)
        ]

        ot_box = {}

        def make_oproj_wave(st, wave, use_act):
            def emit():
                if wave == 0:
                    ot_box[st] = out_pool.tile(
                        [128, HID], BF, tag="outp", name="outp"
                    )
                ot = ot_box[st]
                pw = [pp.tile([128, 512], F32, tag="pp", name="po")
                      for _ in range(2)]
                for hp in range(2):
                    lhsT = at_pair[hp][:, :, st * 128:(st + 1) * 128]
                    for k in range(2):
                        ec = 2 * wave + k
                        nc.tensor.matmul(
                            pw[k], lhsT,
                            wo_sb[hp][:, :, ec * 512:(ec + 1) * 512],
                            start=(hp == 0), stop=(hp == 1), perf_mode=DR,
                        )
                for k in range(2):
                    ec = 2 * wave + k
                    es = slice(ec * 512, (ec + 1) * 512)
                    if use_act and k == 1:
                        nc.scalar.activation(ot[:, es], pw[k], AF.Copy)
                    else:
                        nc.vector.tensor_copy(ot[:, es], pw[k])
                hs = slice(wave * 1024, (wave + 1) * 1024)
                nc.sync.dma_start(out=out[st * 128:(st + 1) * 128, hs],
                                  in_=ot[:, hs])
            return emit

        pending = []
        for sc in range(NSC):
            swin = slice(sc * 512, (sc + 1) * 512)
            ntt = 4 * (sc + 1)
            for hd in range(NHS):
                # all score-pairs of this head first (the pp2 ring lets the
                # PE run up to two pairs ahead of the exp stream), then the
                # pv/den accumulation pairs trail with their exps long done
                ps_at = pp.tile([128, 512], F32, tag="pp", name="at")
                ps_dn = pp.tile([128, 512], F32, tag="pp", name="dn")
                ptps = []
                for ttp in range(ntt // 2):
                    if pending:
                        pending.pop(0)()
                    elif v_rest:
                        # leftover v-projection s-tiles: dense PE filler for
                        # the first chunk's exp latency (no waves exist yet)
                        emit_v(v_rest.pop(0))
                    ptp = pt_pool.tile([128, 2, 512], FP8, tag="probs",
                                       name="probs")
                    ps2 = pp2.tile([128, 1024], F32, tag="pp2", name="ps2")
                    diag = 2 * ttp + 1 - 4 * sc >= 0
                    for i in range(2):
                        tt = 2 * ttp + i
                        j = tt - 4 * sc
                        c0 = 128 * j if j > 0 else 0
                        nc.tensor.matmul(
                            ps2[:, i * 512 + c0:(i + 1) * 512],
                            kts[hd][:, tt * 128:(tt + 1) * 128],
                            qts[hd][:, sc * 512 + c0:(sc + 1) * 512],
                            start=True,
                            stop=True,
                        )
                        if diag:
                            nc.scalar.activation(
                                ptp[:, i, c0:],
                                ps2[:, i * 512 + c0:(i + 1) * 512], AF.Exp,
                            )
                            if c0 > 0:
                                # DR moving operand reads the whole pair tile:
                                # zero the fully-masked region exp never wrote
                                nc.gpsimd.memset(ptp[:, i, 0:c0], 0.0)
                            # keep where (f + c0) - t - 128*j >= 0
                            nc.gpsimd.affine_select(
                                out=ptp[:, i, c0:],
                                in_=ptp[:, i, c0:],
                                compare_op=mybir.AluOpType.is_ge,
                                fill=0.0,
                                base=c0 - 128 * j,
                                channel_multiplier=-1,
                                pattern=[[1, 512 - c0]],
                            )
                    if not diag:
                        # off-diagonal pair: one exp over both PSUM banks
                        nc.scalar.activation(ptp[:, :, :], ps2[:, :], AF.Exp)
                    ptps.append(ptp)
                for ttp, ptp in enumerate(ptps):
                    if ttp % 2 == 1 and pending:
                        pending.pop(0)()
                    nc.tensor.matmul(
                        ps_at,
                        v_sb[ttp][:, :, hd * 128:(hd + 1) * 128],
                        ptp[:, :, :],
                        start=(ttp == 0),
                        stop=(ttp == ntt // 2 - 1),
                        perf_mode=DR,
                    )
                    nc.tensor.matmul(
                        ps_dn,
                        ones8,
                        ptp[:, :, :],
                        start=(ttp == 0),
                        stop=(ttp == ntt // 2 - 1),
                        perf_mode=DR,
                    )
                denb = denb_pool.tile([128, 512], F32, tag="denb", name="denb")
                nc.vector.reciprocal_approx_fast(denb, ps_dn)
                nc.vector.tensor_mul(
                    at_pair[hd // 2][:, hd % 2, swin], ps_at, denb
                )
            for st in range(4 * sc, 4 * sc + 4):
                for wave in range(2):
                    pending.append(make_oproj_wave(st, wave, sc == NSC - 1))
        while pending:
            pending.pop(0)()

    return nc


def get_nc():
    if "nc" not in _STATE:
        nc = _build_nc()
        nc.finalize()
        _STATE["nc"] = nc
    return _STATE["nc"]


def _pack_pairs(a, blk):
    """[2*NP_blk*128, C] -> [NP_blk*128, 2*C]: rows 256p+128i+r -> [p*128+r, i*C+c]."""
    n2, c = a.shape
    npairs = n2 // 256
    a = a.reshape(npairs, 2, 128, c)          # [p, i, r, c]
    a = a.transpose(0, 2, 1, 3)               # [p, r, i, c]
    return np.ascontiguousarray(a.reshape(npairs * 128, 2 * c))


def make_in_maps(x, rms_w, Wq, Wk, Wv, Wo):
    """Host-side sharding: returns one input dict per core (8 cores)."""
    fp8 = ml_dtypes.float8_e4m3fn
    sqk = np.float32(float(HID) ** -0.25)
    rw = rms_w.astype(np.float32)[:, None]
    wq_f = rw * Wq.astype(np.float32) * (sqk * SQK)
    wk_f = rw * Wk.astype(np.float32) * (sqk * SQK)
    wv_f = rw * Wv.astype(np.float32) * SVO
    wo_f = Wo.astype(np.float32) * SVO
    in_maps = []
    for c in range(DP * TP):
        b, i = divmod(c, TP)
        cols = slice(i * KSH, (i + 1) * KSH)
        in_maps.append({
            "xp": _pack_pairs(
                np.ascontiguousarray(x[b].astype(np.float32).T), 128
            ).astype(fp8),
            "wq": _pack_pairs(wq_f[:, cols], 128).astype(fp8),
            "wk": _pack_pairs(wk_f[:, cols], 128).astype(fp8),
            "wv": _pack_pairs(wv_f[:, cols], 128).astype(fp8),
            "wo": _pack_pairs(wo_f[cols, :], 128).astype(fp8),
        })
    return in_maps


def kernel(x, rms_w, Wq, Wk, Wv, Wo, _trace=False, _results_out=None):
    from concourse.bass_utils import run_bass_kernel_spmd

    nc = get_nc()
    in_maps = make_in_maps(x, rms_w, Wq, Wk, Wv, Wo)
    kw = {}
    if _trace:
        kw = dict(trace=True, trace_cores=list(range(DP * TP)))
    res = run_bass_kernel_spmd(
        nc, in_maps, core_ids=list(range(DP * TP)), **kw
    )
    if _results_out is not None:
        _results_out.append(res)
    inv = np.float32(1.0 / SVO)
    out = np.empty((DP, S, HID), np.float32)
    for b in range(DP):
        acc = x[b].astype(np.float32).copy()
        for i in range(TP):
            acc += res.results[b * TP + i]["out"].astype(np.float32) * inv
        out[b] = acc
    return out
